# revision 16
# baseline (speedup 1.0000x reference)
"""MinGRU Trainium2 kernel (B=8, T=8192, D=H=512), SPMD over 8 NeuronCores.

Strategy (v2):
  - Data-parallel over batch: core b computes batch row b end-to-end.
  - Host pre-transposes x[b] -> xT [D, T] and quantizes to fp8 e4m3 (all 512
    d-planes, feeds the k matmul and the first 256 d of the th matmul) plus
    bf16 for d-planes 256..511 (the rest of the th matmul). Rel-err budget
    measured at 1.60e-2 (limit 2e-2) on the fixed jax-key(0) inputs.
  - k = -(Wz q8)^T x8 uses fp8 DoubleRow matmuls: [K=2x128] planes per
    instruction at ~2x bf16 row rate (HW-measured). th = Wh^T x is half
    DoubleRow fp8 / half bf16.
  - ACT: a = sigmoid(-(k+bz)), s = sigmoid(th+bh) (bias/scale fused).
  - GpSimd: g~ = max(th+bh+0.5, s) (scalar_tensor_tensor) -- moved off DVE.
  - DVE: pair-interleaved fused scan (MINGRU_FSCAN2): TWO h-groups'
    independent recurrences alternate element-by-element along the free dim
    via a "p s t -> p t s" access pattern, so the 1-cycle a-flop feedback
    bubble of one stream is filled by the other stream: 1 elem/cycle vs the
    2 cyc/elem single-stream scan. Inits for the two streams ride in as the
    CONST_0/CONST_1 per-partition scalars (uop0/uop1), steady uop self-loops.
  - h is written bf16 (and chunk-chained bf16); host upcasts to f32.
"""

import os
import sys

import numpy as np

if "/opt/trn_rl_repo" not in sys.path:
    sys.path.insert(0, "/opt/trn_rl_repo")

P = 128
B, T, D, H = 8, 8192, 512, 512
GD, GH = D // P, H // P  # 4, 4
TC = 1024  # time chunk; 2-bank PSUM tiles
NCORES = 8

_NC_CACHE = {}
LAST_RESULT = None  # BassKernelResults of the most recent run (for test.py)

_FSCAN2_OP = None


def _fscan2_reference(in0, in1, c0, c1, c2):
    """Numpy reference: two interleaved streams, h[t] = a*h[t-1] + (1-a)*g.
    in0/in1 are [P, 2N] with column 2t+s belonging to stream s."""
    a = np.asarray(in0, np.float32)
    g = np.asarray(in1, np.float32)
    p = a.shape[0]
    fa = a.reshape(p, -1)
    fg = g.reshape(p, -1)

    def init(c):
        if isinstance(c, np.ndarray):
            return c.reshape(p).astype(np.float32).copy()
        return np.full(p, float(c), np.float32)

    st = [init(c0), init(c1)]
    out = np.empty_like(fa)
    for j in range(fa.shape[1]):
        s = j & 1
        st[s] = fa[:, j] * st[s] + (np.float32(1.0) - fa[:, j]) * fg[:, j]
        out[:, j] = st[s]
    return out.reshape(a.shape)


def register_fscan2():
    """Register the packed dual-stream MINGRU_FSCAN2P custom DVE op (TRN2/v3).

    Operands are bf16 [P, 2N]: each 32-bit port read carries one t-step of
    TWO independent streams (column 2t+s = stream s), via the SRC_0/SRC_0_HI
    lane split. One engine element computes BOTH recurrences in the 8 ALU
    blocks ((1-a)*g is computed as g - a*g, which frees the ONE lane):
      blk0: p0 = a0*g0                 blk4: p1 = a1*g1 (captures st0 -> ch0)
      blk1: u0 = g0 - p0               blk5: u1 = g1 - p1
      blk2: m0 = a0*stA  (ch4=i0 1st)  blk6: m1 = a1*stB (ch5=i1 1st;
            captures u0 -> ch1               captures u1 -> ch3)
      blk3: st0 = m0+u0 -> flop A      blk7: st1 = m1+u1 -> flop B
    WR0_LO <- chain0 (st0), WR0_HI <- ALU (st1): one 32-bit write = both
    streams' h as packed bf16. Feedback is 1-element-ago via flops A/B, which
    needs >= 2-cycle element spacing: the steady uop SELF-LOOPS (a self
    transition costs 1 stall cycle, measured), and a bubble uop after the
    first element guarantees the elem0->elem1 gap. Any extra stall only
    widens gaps, so the op is stall-robust. 2 cycles/element = 1 cycle per
    logical (t, stream-pair) step -- 2x the single-stream scan.
    """
    global _FSCAN2_OP
    if _FSCAN2_OP is not None:
        return _FSCAN2_OP

    from concourse.dve_ops import _SUB_OPCODE_FOR_NAME, CUSTOM_DVE_SPECS, OPS, DveOp
    from concourse.dve_spec import One, Spec, Src0, Src1
    from concourse.dve_uop import (
        ENABLE,
        AluInp,
        AluOp,
        DelayInp,
        DveOpSpec,
        InpSel,
        OutPath,
        OutSel,
        Trigger,
        UopConfig,
    )

    if "MINGRU_FSCAN2P" in _SUB_OPCODE_FOR_NAME:
        for op_ in OPS:
            if op_.name == "MINGRU_FSCAN2P":
                _FSCAN2_OP = op_
                return op_

    # placeholder body (never lowered); reference drives CoreSim.
    spec = Spec(body=(One - Src0) * Src1, reference=_fscan2_reference)

    def _compute_uop(first: bool, next_idx: int):
        u = UopConfig()
        # chains 4/5 both carry CONST_0 (=0.0): the real inits ride in as a
        # PREPENDED data element with a=0, g=init (CONST_1/imm1 does not
        # reach the engine on the TTSS encoding -- HW-verified garbage)
        lanes = (
            (1, InpSel.SRC_0),  # a0      -> chain 0
            (2, InpSel.SRC_1),  # g0      -> chain 1
            (3, InpSel.SRC_0_HI),  # a1   -> chain 2
            (4, InpSel.SRC_1_HI),  # g1   -> chain 3
            (5, InpSel.CONST_0),  # 0.0   -> chain 4
            (6, InpSel.CONST_0),  # 0.0   -> chain 5
        )
        for lane, sel in lanes:
            u.inp[lane] = sel
            u.inp_enable[lane] = ENABLE
        dp = u.datapath_config
        # blk0: p0 = a0 * g0
        dp[0].enable_alu(AluOp.MULTIPLY, AluInp.PREV_DELAY_0, AluInp.PREV_DELAY_1)
        dp[0].pass_through_delay(0, 1, 2, 3, 4, 5)
        # blk1: u0 = g0 - p0
        dp[1].enable_alu(AluOp.SUBTRACT, AluInp.PREV_DELAY_1, AluInp.PREV_ALU_OUT)
        dp[1].pass_through_delay(0, 2, 3, 4, 5)
        # blk2: m0 = a0 * state0; capture u0 into chain 1
        st_a = AluInp.PREV_DELAY_4 if first else AluInp.NEXT_ALU_OUT_A
        dp[2].enable_alu(AluOp.MULTIPLY, AluInp.PREV_DELAY_0, st_a)
        dp[2].enable_delay_from_src(DelayInp.PREV_ALU_OUT, 1)
        dp[2].pass_through_delay(2, 3, 5)
        # blk3: st0 = m0 + u0; -> flop A
        dp[3].enable_alu(AluOp.ADD, AluInp.PREV_ALU_OUT, AluInp.PREV_DELAY_1)
        dp[3].alu_out_a_enable = ENABLE
        dp[3].pass_through_delay(2, 3, 5)
        # blk4: p1 = a1 * g1; capture st0 into chain 0
        dp[4].enable_alu(AluOp.MULTIPLY, AluInp.PREV_DELAY_2, AluInp.PREV_DELAY_3)
        dp[4].enable_delay_from_src(DelayInp.PREV_ALU_OUT, 0)
        dp[4].pass_through_delay(2, 3, 5)
        # blk5: u1 = g1 - p1
        dp[5].enable_alu(AluOp.SUBTRACT, AluInp.PREV_DELAY_3, AluInp.PREV_ALU_OUT)
        dp[5].pass_through_delay(0, 2, 5)
        # blk6: m1 = a1 * state1; capture u1 into chain 3
        st_b = AluInp.PREV_DELAY_5 if first else AluInp.NEXT_ALU_OUT_B
        dp[6].enable_alu(AluOp.MULTIPLY, AluInp.PREV_DELAY_2, st_b)
        dp[6].enable_delay_from_src(DelayInp.PREV_ALU_OUT, 3)
        dp[6].pass_through_delay(0)
        # blk7: st1 = m1 + u1; -> flop B
        dp[7].enable_alu(AluOp.ADD, AluInp.PREV_ALU_OUT, AluInp.PREV_DELAY_3)
        dp[7].alu_out_b_enable = ENABLE
        dp[7].pass_through_delay(0)
        u.out[OutPath.WR0_LO] = OutSel.DELAY_0  # st0
        u.out_enable[OutPath.WR0_LO] = ENABLE
        u.out[OutPath.WR0_HI] = OutSel.ALU_OUT  # st1
        u.out_enable[OutPath.WR0_HI] = ENABLE
        u.require_inp0 = ENABLE
        u.require_inp1 = ENABLE
        u.repeat_count = 1
        u.trigger = (Trigger.SRC_TENSOR_DONE, Trigger.COUNT, Trigger.NONE)
        u.next_uop = (0, next_idx, 0)  # done -> idle; else -> next_idx
        return u

    def _bubble_uop(next_idx: int):
        u = UopConfig()
        u.repeat_count = 1
        u.trigger = (Trigger.SRC_TENSOR_DONE, Trigger.COUNT, Trigger.NONE)
        u.next_uop = (0, next_idx, 0)
        return u

    uops = [
        _compute_uop(True, 1),  # elem 0: states <- init0/init1
        _bubble_uop(2),  # guarantee elem0->elem1 spacing >= 2
        _compute_uop(False, 2),  # steady: flops; self-loop (1 stall/elem)
    ]
    for u in uops:
        u.validate("v3")

    row = max(_SUB_OPCODE_FOR_NAME.values()) + 1
    assert row < 0x20

    class _HandDveOp(DveOp):
        def compile(self, ver):
            from concourse.dve_ops import _COMPILE_CACHE

            key = (self.name, ver)
            if key in _COMPILE_CACHE:
                return _COMPILE_CACHE[key]
            assert ver == "v3", "MINGRU_FSCAN2 is hand-authored for TRN2 (v3) only"
            r = DveOpSpec(name=self.name, opcode=row, uops=list(uops), rd1_en=True)
            _COMPILE_CACHE[key] = r
            return r

    op = _HandDveOp(name="MINGRU_FSCAN2", spec=spec, subdim=False, uops_sha={})
    OPS.append(op)
    CUSTOM_DVE_SPECS[op.name] = spec
    _SUB_OPCODE_FOR_NAME[op.name] = row
    _FSCAN2_OP = op
    return op


def _build_nc(t_len=T, tc=TC, thf8=1, hout="bf16", stt_eng="gpsimd"):
    from contextlib import ExitStack

    import concourse.mybir as mybir
    import concourse.tile as tile
    from concourse import bacc

    f32 = mybir.dt.float32
    fp8 = mybir.dt.float8e4
    bf16 = mybir.dt.bfloat16
    f16 = mybir.dt.float16
    Alu = mybir.AluOpType
    Act = mybir.ActivationFunctionType
    DR = mybir.MatmulPerfMode.DoubleRow

    fscan_op = register_fscan2()

    nchunk = t_len // tc
    nh = tc // 2  # x DMA half-chunks
    gdb = GD - 2 * thf8  # bf16 planes for th
    nc = bacc.Bacc("TRN2", target_bir_lowering=False, debug=False)

    xT8 = nc.dram_tensor("xT8", [D, t_len], fp8, kind="ExternalInput").ap()
    wzT8 = nc.dram_tensor("wzT8", [D, H], fp8, kind="ExternalInput").ap()
    whT8 = nc.dram_tensor("whT8", [2 * P * thf8, H], fp8, kind="ExternalInput").ap()
    if gdb:
        xTb = nc.dram_tensor("xTb", [gdb * P, t_len], bf16, kind="ExternalInput").ap()
        whTb = nc.dram_tensor("whTb", [gdb * P, H], bf16, kind="ExternalInput").ap()
    bias3 = nc.dram_tensor("bias3", [P, 3, GH], f32, kind="ExternalInput").ap()
    # packed output: row j*P+p, column c*2*tc + 2*t + s = h for h-group
    # 2*j+s, channel p, time c*tc+t (host de-interleaves)
    hTi = nc.dram_tensor(
        "hTi", [2 * P, nchunk * 2 * tc], f16, kind="ExternalOutput"
    ).ap()

    xT8_g = xT8.rearrange("(g p) t -> p g t", p=P)
    hTi_g = hTi.rearrange("(j p) t -> p j t", p=P)
    if gdb:
        xTb_g = xTb.rearrange("(g p) t -> p g t", p=P)

    with tile.TileContext(nc) as tctx, ExitStack() as ctx:
        singles = ctx.enter_context(tctx.tile_pool(name="singles", bufs=1))
        xpool = ctx.enter_context(tctx.tile_pool(name="xp", bufs=3))
        hpool = ctx.enter_context(tctx.tile_pool(name="hp", bufs=3))
        apool = ctx.enter_context(tctx.tile_pool(name="apool", bufs=3))
        spool = ctx.enter_context(tctx.tile_pool(name="spool", bufs=5))
        gpool = ctx.enter_context(tctx.tile_pool(name="gpool", bufs=3))
        cpool = ctx.enter_context(tctx.tile_pool(name="cpool", bufs=2))
        kp = ctx.enter_context(tctx.tile_pool(name="kp", bufs=2, space="PSUM"))
        tp = ctx.enter_context(tctx.tile_pool(name="tp", bufs=2, space="PSUM"))

        # biases first (tiny DMA, ungates the first ACTIVATE), then weights
        bias3_sb = singles.tile([P, 3, GH], f32)
        nc.scalar.dma_start(out=bias3_sb, in_=bias3)
        wz8_sb = singles.tile([P, GD, H], fp8)
        nc.scalar.dma_start(out=wz8_sb, in_=wzT8.rearrange("(g p) h -> p g h", p=P))
        wh8_sb = singles.tile([P, 2 * thf8, H], fp8)
        nc.scalar.dma_start(out=wh8_sb, in_=whT8.rearrange("(g p) h -> p g h", p=P))
        if gdb:
            whb_sb = singles.tile([P, gdb, H], bf16)
            nc.scalar.dma_start(out=whb_sb, in_=whTb.rearrange("(g p) h -> p g h", p=P))

        stt = nc.gpsimd if stt_eng == "gpsimd" else nc.vector

        h_prev = [None, None]
        c_off = 0
        for c in range(nchunk):
            x8h = []
            if gdb:
                xbc = xpool.tile([P, gdb, tc], bf16, tag="xb")
                nc.sync.dma_start(
                    out=xbc, in_=xTb_g[:, :, c_off : c_off + tc]
                )
            for hidx in range(2):
                tsl = slice(c_off + hidx * nh, c_off + (hidx + 1) * nh)
                x8 = xpool.tile([P, GD, nh], fp8, tag=f"x8_{hidx}")
                nc.sync.dma_start(out=x8, in_=xT8_g[:, :, tsl])
                x8h.append(x8)

            pairs = []
            for j in range(2):  # h-group pairs
                # t-major fp16 interleave: column 2*t+s belongs to stream s --
                # the scan reads flat packed [P, 2*(tc+1)] (SRC_0/SRC_0_HI
                # lane split), ACT/STT write stride-2 fp16 slices [:, 1:, s].
                # Column 0 is the init element: a=0, g=prev chunk's last h
                # (so st = 0*garbage + 1*g = init, robust to flop state).
                a_pair = apool.tile([P, tc + 1, 2], f16, tag=f"a{j}")
                g_pair = gpool.tile([P, tc + 1, 2], f16, tag=f"g{j}")
                nc.gpsimd.memset(a_pair[:, 0, :], 0.0)
                if c == 0:
                    nc.gpsimd.memset(g_pair[:, 0, :], 0.0)
                else:
                    nc.gpsimd.tensor_scalar_add(
                        out=g_pair[:, 0, :], in0=h_prev[j][:, 2 * tc : 2 * tc + 2],
                        scalar1=0.0,
                    )
                for s in range(2):
                    g_ = 2 * j + s
                    gsl = slice(g_ * P, (g_ + 1) * P)
                    kps = kp.tile([P, tc], f32, tag="k")
                    tps = tp.tile([P, tc], f32, tag="t")
                    for ns in range(tc // nh):
                        nsl = slice(ns * nh, (ns + 1) * nh)
                        for jj in range(2):
                            nc.tensor.matmul(
                                kps[:, nsl],
                                wz8_sb[:, 2 * jj : 2 * jj + 2, gsl],
                                x8h[ns][:, 2 * jj : 2 * jj + 2, :],
                                start=(jj == 0),
                                stop=(jj == 1),
                                perf_mode=DR,
                            )
                    for ns in range(tc // nh):
                        nsl = slice(ns * nh, (ns + 1) * nh)
                        for jj in range(thf8):
                            nc.tensor.matmul(
                                tps[:, nsl],
                                wh8_sb[:, 2 * jj : 2 * jj + 2, gsl],
                                x8h[ns][:, 2 * jj : 2 * jj + 2, :],
                                start=(jj == 0),
                                stop=(thf8 == 2 and jj == 1),
                                perf_mode=DR,
                            )
                        for gd in range(gdb):
                            nc.tensor.matmul(
                                tps[:, nsl],
                                whb_sb[:, gd, gsl],
                                xbc[:, gd, nsl],
                                start=False,
                                stop=(gd == gdb - 1),
                            )
                    # a = sigmoid(-(k_mm + bz)) first: k matmuls finish
                    # before th, so the Scalar queue never stalls on th here
                    nc.scalar.activation(
                        out=a_pair[:, 1:, s],
                        in_=kps,
                        func=Act.Sigmoid,
                        bias=bias3_sb[:, 0, g_ : g_ + 1],
                        scale=-1.0,
                    )
                    # s = sigmoid(th_mm + bh)
                    s_sb = spool.tile([P, tc], f32, tag="s")
                    nc.scalar.activation(
                        out=s_sb,
                        in_=tps,
                        func=Act.Sigmoid,
                        bias=bias3_sb[:, 1, g_ : g_ + 1],
                        scale=1.0,
                    )
                    # g~ = max(th_mm + (bh+0.5), s)
                    stt.scalar_tensor_tensor(
                        out=g_pair[:, 1:, s],
                        in0=tps,
                        scalar=bias3_sb[:, 2, g_ : g_ + 1],
                        in1=s_sb,
                        op0=Alu.add,
                        op1=Alu.max,
                    )
                pairs.append((a_pair, g_pair))
            for j in range(2):
                a_pair, g_pair = pairs[j]
                h_pair = hpool.tile([P, 2 * (tc + 1)], f16, tag=f"h{j}")
                nc.vector._custom_dve(
                    fscan_op,
                    out=h_pair[:, :],
                    in0=a_pair[:, :, :].rearrange("p t s -> p (t s)"),
                    in1=g_pair[:, :, :].rearrange("p t s -> p (t s)"),
                    s0=0.0,
                    s1=0.0,
                )
                h_prev[j] = h_pair
                nc.sync.dma_start(
                    out=hTi_g[:, j, 2 * c_off : 2 * (c_off + tc)],
                    in_=h_pair[:, 2:],
                )
            c_off += tc
    nc.compile()
    return nc


def get_nc(t_len=T, tc=TC, thf8=1, hout="bf16", stt_eng="gpsimd"):
    key = (t_len, tc, thf8, hout, stt_eng)
    if key not in _NC_CACHE:
        _NC_CACHE[key] = _build_nc(t_len, tc, thf8, hout, stt_eng)
    return _NC_CACHE[key]


def _prep_shared(Wz, bz, Wh, bh, thf8=1):
    import ml_dtypes

    f = np.float32
    e4 = np.dtype(ml_dtypes.float8_e4m3fn)
    b16 = np.dtype(ml_dtypes.bfloat16)
    df8 = 2 * P * thf8
    shared = {
        "wzT8": np.ascontiguousarray(Wz.T).astype(e4),
        "whT8": np.ascontiguousarray(Wh.T[:df8]).astype(e4),
        "bias3": np.ascontiguousarray(
            np.stack(
                [(-bz).reshape(GH, P).T, bh.reshape(GH, P).T, (bh + 0.5).reshape(GH, P).T],
                axis=1,
            ),
            dtype=f,
        ),
    }
    if df8 < D:
        shared["whTb"] = np.ascontiguousarray(Wh.T[df8:]).astype(b16)
    return shared


def kernel(x, Wz, bz, Wh, bh):
    global LAST_RESULT
    import ml_dtypes

    from concourse import bass_utils

    x = np.asarray(x, dtype=np.float32)
    assert x.shape == (B, T, D), x.shape

    thf8 = int(os.environ.get("MINGRU_THF8", "1"))
    hout = os.environ.get("MINGRU_HOUT", "bf16")
    stt_eng = os.environ.get("MINGRU_STT", "vector")
    nc = get_nc(thf8=thf8, hout=hout, stt_eng=stt_eng)
    e4 = np.dtype(ml_dtypes.float8_e4m3fn)
    b16 = np.dtype(ml_dtypes.bfloat16)
    df8 = 2 * P * thf8
    shared = _prep_shared(
        np.asarray(Wz, np.float32),
        np.asarray(bz, np.float32),
        np.asarray(Wh, np.float32),
        np.asarray(bh, np.float32),
        thf8=thf8,
    )
    in_maps = []
    for b in range(NCORES):
        xt = np.ascontiguousarray(x[b].T)
        m = {"xT8": xt.astype(e4)}
        if df8 < D:
            m["xTb"] = np.ascontiguousarray(xt[df8:]).astype(b16)
        m.update(shared)
        in_maps.append(m)

    res = bass_utils.run_bass_kernel_spmd(
        nc,
        in_maps,
        core_ids=list(range(NCORES)),
        trace=os.environ.get("MINGRU_TRACE", "0") == "1",
    )
    LAST_RESULT = res
    nchunk = T // TC
    out = np.empty((B, T, H), np.float32)
    for b in range(NCORES):
        # [2*P, nchunk*2*TC] packed bf16 -> (j, p, c, t, s) -> [T, H]
        arr = np.asarray(res.results[b]["hTi"]).astype(np.float32)
        arr = arr.reshape(2, P, nchunk, TC, 2)
        out[b] = arr.transpose(2, 3, 0, 4, 1).reshape(T, H)
    return out


# revision 18
# speedup vs baseline: 1.0218x; 1.0218x over previous
"""MinGRU Trainium2 kernel (B=8, T=8192, D=H=512), SPMD over 8 NeuronCores.

Strategy (v2):
  - Data-parallel over batch: core b computes batch row b end-to-end.
  - Host pre-transposes x[b] -> xT [D, T] and quantizes to fp8 e4m3 (all 512
    d-planes, feeds the k matmul and the first 256 d of the th matmul) plus
    bf16 for d-planes 256..511 (the rest of the th matmul). Rel-err budget
    measured at 1.60e-2 (limit 2e-2) on the fixed jax-key(0) inputs.
  - k = -(Wz q8)^T x8 uses fp8 DoubleRow matmuls: [K=2x128] planes per
    instruction at ~2x bf16 row rate (HW-measured). th = Wh^T x is half
    DoubleRow fp8 / half bf16.
  - ACT: a = sigmoid(-(k+bz)), s = sigmoid(th+bh) (bias/scale fused).
  - GpSimd: g~ = max(th+bh+0.5, s) (scalar_tensor_tensor) -- moved off DVE.
  - DVE: pair-interleaved fused scan (MINGRU_FSCAN2): TWO h-groups'
    independent recurrences alternate element-by-element along the free dim
    via a "p s t -> p t s" access pattern, so the 1-cycle a-flop feedback
    bubble of one stream is filled by the other stream: 1 elem/cycle vs the
    2 cyc/elem single-stream scan. Inits for the two streams ride in as the
    CONST_0/CONST_1 per-partition scalars (uop0/uop1), steady uop self-loops.
  - h is written bf16 (and chunk-chained bf16); host upcasts to f32.
"""

import os
import sys

import numpy as np

if "/opt/trn_rl_repo" not in sys.path:
    sys.path.insert(0, "/opt/trn_rl_repo")

P = 128
B, T, D, H = 8, 8192, 512, 512
GD, GH = D // P, H // P  # 4, 4
TC = 1024  # time chunk; 2-bank PSUM tiles
NCORES = 8

_NC_CACHE = {}
LAST_RESULT = None  # BassKernelResults of the most recent run (for test.py)

_FSCAN2_OP = None


def _fscan2_reference(in0, in1, c0, c1, c2):
    """Numpy reference: two interleaved streams, h[t] = a*h[t-1] + (1-a)*g.
    in0/in1 are [P, 2N] with column 2t+s belonging to stream s."""
    a = np.asarray(in0, np.float32)
    g = np.asarray(in1, np.float32)
    p = a.shape[0]
    fa = a.reshape(p, -1)
    fg = g.reshape(p, -1)

    def init(c):
        if isinstance(c, np.ndarray):
            return c.reshape(p).astype(np.float32).copy()
        return np.full(p, float(c), np.float32)

    st = [init(c0), init(c1)]
    out = np.empty_like(fa)
    for j in range(fa.shape[1]):
        s = j & 1
        st[s] = fa[:, j] * st[s] + (np.float32(1.0) - fa[:, j]) * fg[:, j]
        out[:, j] = st[s]
    return out.reshape(a.shape)


def register_fscan2():
    """Register the packed dual-stream MINGRU_FSCAN2P custom DVE op (TRN2/v3).

    Operands are bf16 [P, 2N]: each 32-bit port read carries one t-step of
    TWO independent streams (column 2t+s = stream s), via the SRC_0/SRC_0_HI
    lane split. One engine element computes BOTH recurrences in the 8 ALU
    blocks ((1-a)*g is computed as g - a*g, which frees the ONE lane):
      blk0: p0 = a0*g0                 blk4: p1 = a1*g1 (captures st0 -> ch0)
      blk1: u0 = g0 - p0               blk5: u1 = g1 - p1
      blk2: m0 = a0*stA  (ch4=i0 1st)  blk6: m1 = a1*stB (ch5=i1 1st;
            captures u0 -> ch1               captures u1 -> ch3)
      blk3: st0 = m0+u0 -> flop A      blk7: st1 = m1+u1 -> flop B
    WR0_LO <- chain0 (st0), WR0_HI <- ALU (st1): one 32-bit write = both
    streams' h as packed bf16. Feedback is 1-element-ago via flops A/B, which
    needs >= 2-cycle element spacing: the steady uop SELF-LOOPS (a self
    transition costs 1 stall cycle, measured), and a bubble uop after the
    first element guarantees the elem0->elem1 gap. Any extra stall only
    widens gaps, so the op is stall-robust. 2 cycles/element = 1 cycle per
    logical (t, stream-pair) step -- 2x the single-stream scan.
    """
    global _FSCAN2_OP
    if _FSCAN2_OP is not None:
        return _FSCAN2_OP

    from concourse.dve_ops import _SUB_OPCODE_FOR_NAME, CUSTOM_DVE_SPECS, OPS, DveOp
    from concourse.dve_spec import One, Spec, Src0, Src1
    from concourse.dve_uop import (
        ENABLE,
        AluInp,
        AluOp,
        DelayInp,
        DveOpSpec,
        InpSel,
        OutPath,
        OutSel,
        Trigger,
        UopConfig,
    )

    if "MINGRU_FSCAN2P" in _SUB_OPCODE_FOR_NAME:
        for op_ in OPS:
            if op_.name == "MINGRU_FSCAN2P":
                _FSCAN2_OP = op_
                return op_

    # placeholder body (never lowered); reference drives CoreSim.
    spec = Spec(body=(One - Src0) * Src1, reference=_fscan2_reference)

    def _compute_uop(first: bool, next_idx: int):
        u = UopConfig()
        # chains 4/5 both carry CONST_0 (=0.0): the real inits ride in as a
        # PREPENDED data element with a=0, g=init (CONST_1/imm1 does not
        # reach the engine on the TTSS encoding -- HW-verified garbage)
        lanes = (
            (1, InpSel.SRC_0),  # a0      -> chain 0
            (2, InpSel.SRC_1),  # g0      -> chain 1
            (3, InpSel.SRC_0_HI),  # a1   -> chain 2
            (4, InpSel.SRC_1_HI),  # g1   -> chain 3
            (5, InpSel.CONST_0),  # 0.0   -> chain 4
            (6, InpSel.CONST_0),  # 0.0   -> chain 5
        )
        for lane, sel in lanes:
            u.inp[lane] = sel
            u.inp_enable[lane] = ENABLE
        dp = u.datapath_config
        # blk0: p0 = a0 * g0
        dp[0].enable_alu(AluOp.MULTIPLY, AluInp.PREV_DELAY_0, AluInp.PREV_DELAY_1)
        dp[0].pass_through_delay(0, 1, 2, 3, 4, 5)
        # blk1: u0 = g0 - p0
        dp[1].enable_alu(AluOp.SUBTRACT, AluInp.PREV_DELAY_1, AluInp.PREV_ALU_OUT)
        dp[1].pass_through_delay(0, 2, 3, 4, 5)
        # blk2: m0 = a0 * state0; capture u0 into chain 1
        st_a = AluInp.PREV_DELAY_4 if first else AluInp.NEXT_ALU_OUT_A
        dp[2].enable_alu(AluOp.MULTIPLY, AluInp.PREV_DELAY_0, st_a)
        dp[2].enable_delay_from_src(DelayInp.PREV_ALU_OUT, 1)
        dp[2].pass_through_delay(2, 3, 5)
        # blk3: st0 = m0 + u0; -> flop A
        dp[3].enable_alu(AluOp.ADD, AluInp.PREV_ALU_OUT, AluInp.PREV_DELAY_1)
        dp[3].alu_out_a_enable = ENABLE
        dp[3].pass_through_delay(2, 3, 5)
        # blk4: p1 = a1 * g1; capture st0 into chain 0
        dp[4].enable_alu(AluOp.MULTIPLY, AluInp.PREV_DELAY_2, AluInp.PREV_DELAY_3)
        dp[4].enable_delay_from_src(DelayInp.PREV_ALU_OUT, 0)
        dp[4].pass_through_delay(2, 3, 5)
        # blk5: u1 = g1 - p1
        dp[5].enable_alu(AluOp.SUBTRACT, AluInp.PREV_DELAY_3, AluInp.PREV_ALU_OUT)
        dp[5].pass_through_delay(0, 2, 5)
        # blk6: m1 = a1 * state1; capture u1 into chain 3
        st_b = AluInp.PREV_DELAY_5 if first else AluInp.NEXT_ALU_OUT_B
        dp[6].enable_alu(AluOp.MULTIPLY, AluInp.PREV_DELAY_2, st_b)
        dp[6].enable_delay_from_src(DelayInp.PREV_ALU_OUT, 3)
        dp[6].pass_through_delay(0)
        # blk7: st1 = m1 + u1; -> flop B
        dp[7].enable_alu(AluOp.ADD, AluInp.PREV_ALU_OUT, AluInp.PREV_DELAY_3)
        dp[7].alu_out_b_enable = ENABLE
        dp[7].pass_through_delay(0)
        u.out[OutPath.WR0_LO] = OutSel.DELAY_0  # st0
        u.out_enable[OutPath.WR0_LO] = ENABLE
        u.out[OutPath.WR0_HI] = OutSel.ALU_OUT  # st1
        u.out_enable[OutPath.WR0_HI] = ENABLE
        u.require_inp0 = ENABLE
        u.require_inp1 = ENABLE
        u.repeat_count = 1
        u.trigger = (Trigger.SRC_TENSOR_DONE, Trigger.COUNT, Trigger.NONE)
        u.next_uop = (0, next_idx, 0)  # done -> idle; else -> next_idx
        return u

    def _bubble_uop(next_idx: int):
        u = UopConfig()
        u.repeat_count = 1
        u.trigger = (Trigger.SRC_TENSOR_DONE, Trigger.COUNT, Trigger.NONE)
        u.next_uop = (0, next_idx, 0)
        return u

    uops = [
        _compute_uop(True, 1),  # elem 0: states <- init0/init1
        _bubble_uop(2),  # guarantee elem0->elem1 spacing >= 2
        _compute_uop(False, 2),  # steady: flops; self-loop (1 stall/elem)
    ]
    for u in uops:
        u.validate("v3")

    row = max(_SUB_OPCODE_FOR_NAME.values()) + 1
    assert row < 0x20

    class _HandDveOp(DveOp):
        def compile(self, ver):
            from concourse.dve_ops import _COMPILE_CACHE

            key = (self.name, ver)
            if key in _COMPILE_CACHE:
                return _COMPILE_CACHE[key]
            assert ver == "v3", "MINGRU_FSCAN2 is hand-authored for TRN2 (v3) only"
            r = DveOpSpec(name=self.name, opcode=row, uops=list(uops), rd1_en=True)
            _COMPILE_CACHE[key] = r
            return r

    op = _HandDveOp(name="MINGRU_FSCAN2", spec=spec, subdim=False, uops_sha={})
    OPS.append(op)
    CUSTOM_DVE_SPECS[op.name] = spec
    _SUB_OPCODE_FOR_NAME[op.name] = row
    _FSCAN2_OP = op
    return op


def _build_nc(t_len=T, tc=TC, thf8=1, hout="bf16", stt_eng="gpsimd"):
    from contextlib import ExitStack

    import concourse.mybir as mybir
    import concourse.tile as tile
    from concourse import bacc

    f32 = mybir.dt.float32
    fp8 = mybir.dt.float8e4
    bf16 = mybir.dt.bfloat16
    f16 = mybir.dt.float16
    Alu = mybir.AluOpType
    Act = mybir.ActivationFunctionType
    DR = mybir.MatmulPerfMode.DoubleRow

    fscan_op = register_fscan2()

    nchunk = t_len // tc
    nh = tc // 2  # x DMA half-chunks
    gdb = GD - 2 * thf8  # bf16 planes for th
    nc = bacc.Bacc("TRN2", target_bir_lowering=False, debug=False)

    xT8 = nc.dram_tensor("xT8", [D, t_len], fp8, kind="ExternalInput").ap()
    wzT8 = nc.dram_tensor("wzT8", [D, H], fp8, kind="ExternalInput").ap()
    whT8 = nc.dram_tensor("whT8", [2 * P * thf8, H], fp8, kind="ExternalInput").ap()
    if gdb:
        xTb = nc.dram_tensor("xTb", [gdb * P, t_len], bf16, kind="ExternalInput").ap()
        whTb = nc.dram_tensor("whTb", [gdb * P, H], bf16, kind="ExternalInput").ap()
    bias3 = nc.dram_tensor("bias3", [P, 3, GH], f32, kind="ExternalInput").ap()
    # packed output: row j*P+p, column c*2*tc + 2*t + s = h for h-group
    # 2*j+s, channel p, time c*tc+t (host de-interleaves)
    hTi = nc.dram_tensor(
        "hTi", [2 * P, nchunk * 2 * tc], f16, kind="ExternalOutput"
    ).ap()

    xT8_g = xT8.rearrange("(g p) t -> p g t", p=P)
    hTi_g = hTi.rearrange("(j p) t -> p j t", p=P)
    if gdb:
        xTb_g = xTb.rearrange("(g p) t -> p g t", p=P)

    with tile.TileContext(nc) as tctx, ExitStack() as ctx:
        singles = ctx.enter_context(tctx.tile_pool(name="singles", bufs=1))
        xpool = ctx.enter_context(tctx.tile_pool(name="xp", bufs=3))
        hpool = ctx.enter_context(tctx.tile_pool(name="hp", bufs=3))
        apool = ctx.enter_context(tctx.tile_pool(name="apool", bufs=3))
        spool = ctx.enter_context(tctx.tile_pool(name="spool", bufs=5))
        gpool = ctx.enter_context(tctx.tile_pool(name="gpool", bufs=3))
        cpool = ctx.enter_context(tctx.tile_pool(name="cpool", bufs=2))
        kp = ctx.enter_context(tctx.tile_pool(name="kp", bufs=2, space="PSUM"))
        tp = ctx.enter_context(tctx.tile_pool(name="tp", bufs=2, space="PSUM"))

        # biases first (tiny DMA, ungates the first ACTIVATE); weight DMAs
        # spread across queues so they run in parallel with each other and
        # with the first x chunk
        bias3_sb = singles.tile([P, 3, GH], f32)
        nc.scalar.dma_start(out=bias3_sb, in_=bias3)
        wz8_sb = singles.tile([P, GD, H], fp8)
        nc.scalar.dma_start(out=wz8_sb, in_=wzT8.rearrange("(g p) h -> p g h", p=P))
        wh8_sb = singles.tile([P, 2 * thf8, H], fp8)
        nc.gpsimd.dma_start(out=wh8_sb, in_=whT8.rearrange("(g p) h -> p g h", p=P))
        if gdb:
            whb_sb = singles.tile([P, gdb, H], bf16)
            nc.sync.dma_start(out=whb_sb, in_=whTb.rearrange("(g p) h -> p g h", p=P))

        stt = nc.gpsimd if stt_eng == "gpsimd" else nc.vector

        h_prev = [None, None]
        prev_n = 0
        c_off = 0
        chunk_sizes = [tc] * (nchunk - 1) + [tc // 2, tc // 2]
        for c in range(len(chunk_sizes)):
            tcc = chunk_sizes[c]
            nhh = min(nh, tcc)
            x8h = []
            if gdb:
                xbc = xpool.tile([P, gdb, tcc], bf16, tag="xb")
                nc.sync.dma_start(
                    out=xbc, in_=xTb_g[:, :, c_off : c_off + tcc]
                )
            for hidx in range(tcc // nhh):
                tsl = slice(c_off + hidx * nhh, c_off + (hidx + 1) * nhh)
                x8 = xpool.tile([P, GD, nhh], fp8, tag=f"x8_{hidx}")
                nc.sync.dma_start(out=x8, in_=xT8_g[:, :, tsl])
                x8h.append(x8)

            pairs = []
            for j in range(2):  # h-group pairs
                # t-major fp16 interleave: column 2*t+s belongs to stream s --
                # the scan reads flat packed [P, 2*(tc+1)] (SRC_0/SRC_0_HI
                # lane split), ACT/STT write stride-2 fp16 slices [:, 1:, s].
                # Column 0 is the init element: a=0, g=prev chunk's last h
                # (so st = 0*garbage + 1*g = init, robust to flop state).
                a_pair = apool.tile([P, tcc + 1, 2], f16, tag=f"a{j}")
                g_pair = gpool.tile([P, tcc + 1, 2], f16, tag=f"g{j}")
                nc.gpsimd.memset(a_pair[:, 0, :], 0.0)
                if c == 0:
                    nc.gpsimd.memset(g_pair[:, 0, :], 0.0)
                else:
                    nc.gpsimd.tensor_scalar_add(
                        out=g_pair[:, 0, :], in0=h_prev[j][:, prev_n : prev_n + 2],
                        scalar1=0.0,
                    )
                for s in range(2):
                    g_ = 2 * j + s
                    gsl = slice(g_ * P, (g_ + 1) * P)
                    kps = kp.tile([P, tcc], f32, tag="k")
                    tps = tp.tile([P, tcc], f32, tag="t")
                    for ns in range(tcc // nhh):
                        nsl = slice(ns * nhh, (ns + 1) * nhh)
                        for jj in range(2):
                            nc.tensor.matmul(
                                kps[:, nsl],
                                wz8_sb[:, 2 * jj : 2 * jj + 2, gsl],
                                x8h[ns][:, 2 * jj : 2 * jj + 2, :],
                                start=(jj == 0),
                                stop=(jj == 1),
                                perf_mode=DR,
                            )
                    for ns in range(tcc // nhh):
                        nsl = slice(ns * nhh, (ns + 1) * nhh)
                        for jj in range(thf8):
                            nc.tensor.matmul(
                                tps[:, nsl],
                                wh8_sb[:, 2 * jj : 2 * jj + 2, gsl],
                                x8h[ns][:, 2 * jj : 2 * jj + 2, :],
                                start=(jj == 0),
                                stop=(thf8 == 2 and jj == 1),
                                perf_mode=DR,
                            )
                        for gd in range(gdb):
                            nc.tensor.matmul(
                                tps[:, nsl],
                                whb_sb[:, gd, gsl],
                                xbc[:, gd, nsl],
                                start=False,
                                stop=(gd == gdb - 1),
                            )
                    # a = sigmoid(-(k_mm + bz)) first: k matmuls finish
                    # before th, so the Scalar queue never stalls on th here
                    nc.scalar.activation(
                        out=a_pair[:, 1:, s],
                        in_=kps,
                        func=Act.Sigmoid,
                        bias=bias3_sb[:, 0, g_ : g_ + 1],
                        scale=-1.0,
                    )
                    # s = sigmoid(th_mm + bh)
                    s_sb = spool.tile([P, tcc], f32, tag="s")
                    nc.scalar.activation(
                        out=s_sb,
                        in_=tps,
                        func=Act.Sigmoid,
                        bias=bias3_sb[:, 1, g_ : g_ + 1],
                        scale=1.0,
                    )
                    # g~ = max(th_mm + (bh+0.5), s)
                    stt.scalar_tensor_tensor(
                        out=g_pair[:, 1:, s],
                        in0=tps,
                        scalar=bias3_sb[:, 2, g_ : g_ + 1],
                        in1=s_sb,
                        op0=Alu.add,
                        op1=Alu.max,
                    )
                pairs.append((a_pair, g_pair))
            for j in range(2):
                a_pair, g_pair = pairs[j]
                h_pair = hpool.tile([P, 2 * (tcc + 1)], f16, tag=f"h{j}")
                nc.vector._custom_dve(
                    fscan_op,
                    out=h_pair[:, :],
                    in0=a_pair[:, :, :].rearrange("p t s -> p (t s)"),
                    in1=g_pair[:, :, :].rearrange("p t s -> p (t s)"),
                    s0=0.0,
                    s1=0.0,
                )
                h_prev[j] = h_pair
                nc.sync.dma_start(
                    out=hTi_g[:, j, 2 * c_off : 2 * (c_off + tcc)],
                    in_=h_pair[:, 2:],
                )
            c_off += tcc
            prev_n = 2 * tcc
    nc.compile()
    return nc


def get_nc(t_len=T, tc=TC, thf8=1, hout="bf16", stt_eng="gpsimd"):
    key = (t_len, tc, thf8, hout, stt_eng)
    if key not in _NC_CACHE:
        _NC_CACHE[key] = _build_nc(t_len, tc, thf8, hout, stt_eng)
    return _NC_CACHE[key]


def _prep_shared(Wz, bz, Wh, bh, thf8=1):
    import ml_dtypes

    f = np.float32
    e4 = np.dtype(ml_dtypes.float8_e4m3fn)
    b16 = np.dtype(ml_dtypes.bfloat16)
    df8 = 2 * P * thf8
    shared = {
        "wzT8": np.ascontiguousarray(Wz.T).astype(e4),
        "whT8": np.ascontiguousarray(Wh.T[:df8]).astype(e4),
        "bias3": np.ascontiguousarray(
            np.stack(
                [(-bz).reshape(GH, P).T, bh.reshape(GH, P).T, (bh + 0.5).reshape(GH, P).T],
                axis=1,
            ),
            dtype=f,
        ),
    }
    if df8 < D:
        shared["whTb"] = np.ascontiguousarray(Wh.T[df8:]).astype(b16)
    return shared


def kernel(x, Wz, bz, Wh, bh):
    global LAST_RESULT
    import ml_dtypes

    from concourse import bass_utils

    x = np.asarray(x, dtype=np.float32)
    assert x.shape == (B, T, D), x.shape

    thf8 = int(os.environ.get("MINGRU_THF8", "1"))
    hout = os.environ.get("MINGRU_HOUT", "bf16")
    stt_eng = os.environ.get("MINGRU_STT", "vector")
    nc = get_nc(thf8=thf8, hout=hout, stt_eng=stt_eng)
    e4 = np.dtype(ml_dtypes.float8_e4m3fn)
    b16 = np.dtype(ml_dtypes.bfloat16)
    df8 = 2 * P * thf8
    shared = _prep_shared(
        np.asarray(Wz, np.float32),
        np.asarray(bz, np.float32),
        np.asarray(Wh, np.float32),
        np.asarray(bh, np.float32),
        thf8=thf8,
    )
    in_maps = []
    for b in range(NCORES):
        xt = np.ascontiguousarray(x[b].T)
        m = {"xT8": xt.astype(e4)}
        if df8 < D:
            m["xTb"] = np.ascontiguousarray(xt[df8:]).astype(b16)
        m.update(shared)
        in_maps.append(m)

    res = bass_utils.run_bass_kernel_spmd(
        nc,
        in_maps,
        core_ids=list(range(NCORES)),
        trace=os.environ.get("MINGRU_TRACE", "0") == "1",
    )
    LAST_RESULT = res
    nchunk = T // TC
    out = np.empty((B, T, H), np.float32)
    for b in range(NCORES):
        # [2*P, nchunk*2*TC] packed bf16 -> (j, p, c, t, s) -> [T, H]
        arr = np.asarray(res.results[b]["hTi"]).astype(np.float32)
        arr = arr.reshape(2, P, nchunk, TC, 2)
        out[b] = arr.transpose(2, 3, 0, 4, 1).reshape(T, H)
    return out
